# revision 24
# baseline (speedup 1.0000x reference)
"""Trainium2 Bass kernel for RAFT-style CorrBlock (all-pairs correlation +
pyramid + 9x9 bilinear window sampling).

Contract: kernel(**inputs) takes FULL inputs (fmap1, fmap2, centroids_coords)
and returns the FULL output (B, NUM_LEVELS*81, H, W) as float32.

Strategy (v2 -- pixel-group band windows + fused level chains)
--------------------------------------------------------------
* avg-pooling the correlation volume == correlating against avg-pooled fmap2
  -> pool fmap2 on host, never materialize the (BHW, H, W) pyramid.
* pixels are sorted by centroid-y into 18 tiles of 128, then by centroid-x
  within each tile.  Per tile the x-sorted pixels are split into pixel
  GROUPS (2x64 at L0, 4x32 at L1); each group gets its own narrow band
  window of (pooled) fmap2, gathered on the host.  Col-tiled matmuls
  (tile_position=(0, 32g/64g)) write every group's correlation band to the
  SAME psum columns on the group's own partitions -- so the per-pixel
  coarse-x select shrinks from a 96-wide radix-16+2 cascade to a short
  direct radix-8 masked chain over an 18-wide window.  L2/L3 maps are small
  enough that a shared full-width band is cheaper.
* all selection/blend stages are uniform 18-wide masked tensor_tensor
  chains batched over up-to-6-tile groups; the y-select and the fine-x
  (q/j) stages are additionally fused ACROSS PYRAMID LEVELS via a
  slot-major [tile-member x level] layout, cutting DVE instruction count
  ~2.4x vs per-level chains (the ~151-cycle per-op DVE overhead was ~30%
  of the baseline's runtime).
* engine split: TensorE runs the col-tiled band matmuls; ACT copies
  PSUM->SBUF band slabs (with the fp32->bf16 cast); DVE runs every
  select/blend chain (gpsimd is kept idle: its SBUF port is shared with
  the DVE and any gpsimd streaming halves DVE throughput); Sync drives
  all DMAs.
"""

import os
import sys
import types

import numpy as np

if "/opt/trn_rl_repo" not in sys.path:
    sys.path.insert(0, "/opt/trn_rl_repo")

import ml_dtypes

BF16 = ml_dtypes.bfloat16

# ----------------------------------------------------------------- constants
B, C, H, W = 2, 256, 96, 96
NUM_LEVELS = 4
RADIUS = 4
K = 2 * RADIUS + 1  # 9
HW = H * W
NCORES = 8
P_CORE = B * HW // NCORES  # 2304 query pixels per core
TP = 128                   # pixels (partitions) per tile
TILES = P_CORE // TP       # 18
G = 6                      # max tiles per batched select group

WL = [96, 48, 24, 12]      # level map widths (== heights)
SPAN = [2, 1, 1, 1]        # max allowed y0 span inside a tile
BH = [12, 11, 11, 11]      # band rows per level
NY = [4, 3, 3, 3]          # y-select taps per level

GPX = [64, 32, 128, 128]   # pixels per x-group (128 => shared band)
WIN = [70, 36, 42, 30]     # band window width per level
NA = [7, 3, 3, 2]          # direct radix-8 select options per level
NL = [BH[l] * WIN[l] for l in range(NUM_LEVELS)]  # [840, 396, 462, 330]

# f2band column layout per (tile, cc): [L1 x4 | L0 x2 | L2 | L3]
C_L1 = [0, 396, 792, 1188]
C_L0 = [1584, 2424]
C_L2 = 3264
C_L3 = 3726
NCOLS = 4056

# psum: psA [128,1352->1536 x2bufs]: L1 [0:396), L0 [512:1352)
#       psB [128,842->1024 x1]: L2 [0:462), L3 [512:842)
# repS column layout (stage-1 one-hots + L0 y-coefs), 18-wide runs:
S_A = [0, 126, 180, 234]   # L0 7x18, L1 3x18, L2 3x18, L3 2x18
S_Y0 = 270                 # L0 y: 4x18
RWS = 342
# repY: slot (t, l-1), 3 y taps x 18 = 54
# repQJ: slot (t, l), q 3x12 | j 4x9 = 72

GROUPS = [(0, 2), (2, 4), (6, 6), (12, 6)]

_cached = {}


# ------------------------------------------------------------------- helpers
def _pool_levels(f2_scaled):
    """f2_scaled: (C, H, W) fp32 -> list of (C, H_l, W_l)."""
    out = [f2_scaled]
    cur = f2_scaled
    for _ in range(NUM_LEVELS - 1):
        c, h, w = cur.shape
        cur = cur.reshape(c, h // 2, 2, w // 2, 2).mean(axis=(2, 4), dtype=np.float32)
        out.append(cur)
    return out


def _sample_np(cmap, cx, cy):
    """Reference-equivalent 9x9 bilinear sampling of one level map.

    cmap: (n, h, w) fp32; cx, cy: (n,) absolute coords at this level.
    Returns (n, K, K) with [i, j] = sample at (x=cx+di[i], y=cy+di[j]).
    """
    n, h, w = cmap.shape
    di = np.linspace(-RADIUS, RADIUS, K).astype(np.float32)
    x = np.broadcast_to(cx[:, None, None] + di[None, :, None], (n, K, K))
    y = np.broadcast_to(cy[:, None, None] + di[None, None, :], (n, K, K))
    x0 = np.floor(x)
    y0 = np.floor(y)
    wx1 = x - x0
    wy1 = y - y0
    res = np.zeros((n, K, K), np.float32)
    ni = np.arange(n)[:, None, None]
    for dx, wxt in ((0, 1.0 - wx1), (1, wx1)):
        for dy, wyt in ((0, 1.0 - wy1), (1, wy1)):
            xi = x0 + dx
            yi = y0 + dy
            valid = (xi >= 0) & (xi <= w - 1) & (yi >= 0) & (yi <= h - 1)
            xc = np.clip(xi, 0, w - 1).astype(np.int64)
            yc = np.clip(yi, 0, h - 1).astype(np.int64)
            res += np.where(valid, cmap[ni, yc, xc], 0.0) * wxt * wyt
    return res


# ------------------------------------------------------------- bass program
def _build_program():
    import concourse.bass as bass
    import concourse.tile as tile
    from concourse import mybir
    from concourse.vector_clock import ScopedClock

    # walrus in this container only supports one sync wait on the tail
    # Drain/NoOp -- split the tile tail waits onto single-wait NOPs.
    def _patched_drain_and_barrier(self, tick_clock, wait_clock):
        nc = self.nc
        probe = nc.sync.nop()
        wait_clock.add_sem_waits(probe.ins, ScopedClock({None: tick_clock.global_clock}))
        si = probe.ins.sync_info
        waits = list(si.on_wait or []) if si else []
        if len(waits) > 1:
            si.on_wait = waits[:1]
            for wt in waits[1:]:
                n2 = nc.sync.nop()
                n2.ins.sync_info = mybir.SyncInfo(on_wait=[wt], on_update=[])
        nc.sync.drain()
        nc.all_engine_barrier()
        popped = nc._tile_sem_poison_stack.pop()
        assert popped is self._sem_poison
        nc.clear_and_free_semaphores(list(self.sems.allocated().values()))
        nc.all_engine_barrier()

    tile.TileContext._drain_and_barrier = _patched_drain_and_barrier

    f32 = mybir.dt.float32
    bf16 = mybir.dt.bfloat16
    MUL = mybir.AluOpType.mult
    ADD = mybir.AluOpType.add


    nc = bass.Bass()
    f1_h = nc.declare_dram_parameter("f1p", [2, 128, P_CORE], bf16, isOutput=False)
    f2_h = nc.declare_dram_parameter("f2band", [TILES, 2, 128, NCOLS], bf16,
                                     isOutput=False)
    repS_h = nc.declare_dram_parameter("repS", [128, TILES, 1, RWS], bf16,
                                       isOutput=False)
    repY_h = nc.declare_dram_parameter("repY", [128, TILES * 3, 1, 54], bf16,
                                       isOutput=False)
    repQJ_h = nc.declare_dram_parameter("repQJ", [128, TILES * 4, 1, 72], bf16,
                                        isOutput=False)
    out_h = nc.declare_dram_parameter(
        "out", [128, TILES * 4, 81], bf16, isOutput=True)

    with tile.TileContext(nc) as tc:
        with (
            tc.tile_pool(name="persist", bufs=1) as persist,
            tc.tile_pool(name="f2in", bufs=2) as f2in,
            tc.tile_pool(name="psum", bufs=1, space="PSUM") as psumpool,
            tc.tile_pool(name="outp", bufs=2) as outp,
        ):
            f1sb = [persist.tile([128, P_CORE], bf16, tag=f"f1_{cc}", name=f"f1_{cc}")
                    for cc in range(2)]
            for cc in range(2):
                nc.sync.dma_start(f1sb[cc][:], f1_h[cc])
            repSsb = persist.tile([128, TILES, 1, RWS], bf16, tag="repS", name="repS")
            repYsb = persist.tile([128, TILES * 3, 1, 54], bf16, tag="repY", name="repY")
            repQJsb = persist.tile([128, TILES * 4, 1, 72], bf16, tag="repQJ",
                                   name="repQJ")


            # band slabs, double-buffered per group parity; each slab is a
            # verbatim image of its psum tile (incl. the 512-alignment gaps)
            # so one ACT copy moves a whole tile: A = [L1 0:396 | L0 512:1352),
            # B = [L2 0:462 | L3 512:842)
            bandA = [persist.tile([128, G, 1352], bf16, tag=f"bandA_{pp}",
                                  name=f"bandA_{pp}") for pp in range(2)]
            bandB = [persist.tile([128, G, 842], bf16, tag=f"bandB_{pp}",
                                  name=f"bandB_{pp}") for pp in range(2)]

            def band_view(l, pp):
                if l == 0:
                    return bandA[pp][:, :, 512:1352].rearrange(
                        "p g (r w) -> p g r w", w=70)
                if l == 1:
                    return bandA[pp][:, :, 0:396].rearrange(
                        "p g (r w) -> p g r w", w=36)
                if l == 2:
                    return bandB[pp][:, :, 0:462].rearrange(
                        "p g (r w) -> p g r w", w=42)
                return bandB[pp][:, :, 512:842].rearrange(
                    "p g (r w) -> p g r w", w=30)
            # stage-1 outputs
            s1fL0 = persist.tile([128, G, 12, 18], bf16, tag="s1fL0", name="s1fL0")
            s1f123 = [persist.tile([128, G * 3, 11, 18], bf16, tag=f"s1f123_{pp}",
                                   name=f"s1f123_{pp}") for pp in range(2)]
            s2m = persist.tile([128, G * 4, 9, 18], bf16, tag="s2m", name="s2m")
            # scratch: L0 select (also reused, row-sliced, by the L0 y chain)
            pS = [persist.tile([128, G, 12, 18], bf16, tag=f"pS{i}", name=f"pS{i}")
                  for i in range(3)]
            # y123 scratch
            wv = [persist.tile([128, G * 3, 9, 18], bf16, tag=f"wv{i}", name=f"wv{i}")
                  for i in range(3)]
            # q scratch (also reused, col-sliced, by the j chain)
            qv = [persist.tile([128, G * 4, 9, 12], bf16, tag=f"qv{i}", name=f"qv{i}")
                  for i in range(3)]

            tile_grp = {}
            for grp, (gs, gn) in enumerate(GROUPS):
                for i in range(gn):
                    tile_grp[gs + i] = (grp, gs, gn, i)

            def coefS(gs, gn, col, rows, w=18):
                return repSsb[:, gs:gs + gn, 0:1, col:col + w].broadcast_to(
                    (128, gn, rows, w))

            def coefY(gs, gn, col):
                return repYsb[:, gs * 3:(gs + gn) * 3, 0:1, col:col + 18].broadcast_to(
                    (128, 3 * gn, 9, 18))

            def coefQJ(gs, gn, col, w):
                return repQJsb[:, gs * 4:(gs + gn) * 4, 0:1, col:col + w].broadcast_to(
                    (128, 4 * gn, 9, w))

            for t in range(TILES):
                grp, gs, gn, gi = tile_grp[t]
                pp = grp % 2
                f2sb = [f2in.tile([128, NCOLS], bf16, tag=f"f2_{cc}",
                                  name=f"f2sb_{cc}") for cc in range(2)]
                for cc in range(2):
                    nc.sync.dma_start(f2sb[cc][:], f2_h[t, cc])
                if t == 0:
                    nc.sync.dma_start(repSsb[:], repS_h[:])
                elif t == 1:
                    nc.sync.dma_start(repYsb[:], repY_h[:])
                    nc.sync.dma_start(repQJsb[:], repQJ_h[:])

                psA = psumpool.tile([128, 1352], f32, tag="psA", name="psA", bufs=2)
                psB = psumpool.tile([128, 842], f32, tag="psB", name="psB")
                for cc in range(2):
                    st = (cc == 0)
                    sp = (cc == 1)
                    for g in range(4):  # L1 groups of 32
                        lhsT = f1sb[cc][:, t * TP + g * 32:t * TP + (g + 1) * 32]
                        nc.tensor.matmul(
                            psA[g * 32:(g + 1) * 32, 0:396], lhsT,
                            f2sb[cc][:, C_L1[g]:C_L1[g] + 396],
                            start=st, stop=sp, tile_position=(0, g * 32))
                    for g in range(2):  # L0 groups of 64
                        lhsT = f1sb[cc][:, t * TP + g * 64:t * TP + (g + 1) * 64]
                        nc.tensor.matmul(
                            psA[g * 64:(g + 1) * 64, 512:1024], lhsT,
                            f2sb[cc][:, C_L0[g]:C_L0[g] + 512],
                            start=st, stop=sp, tile_position=(0, g * 64))
                        nc.tensor.matmul(
                            psA[g * 64:(g + 1) * 64, 1024:1352], lhsT,
                            f2sb[cc][:, C_L0[g] + 512:C_L0[g] + 840],
                            start=st, stop=sp, tile_position=(0, g * 64))
                    lhsT = f1sb[cc][:, t * TP:(t + 1) * TP]
                    nc.tensor.matmul(psB[:, 0:462], lhsT,
                                     f2sb[cc][:, C_L2:C_L2 + 462],
                                     start=st, stop=sp)
                    nc.tensor.matmul(psB[:, 512:842], lhsT,
                                     f2sb[cc][:, C_L3:C_L3 + 330],
                                     start=st, stop=sp)

                # PSUM -> SBUF band slabs (fp32 -> bf16 on ACT), one copy
                # per psum tile
                nc.scalar.copy(bandA[pp][:, gi], psA[:])
                nc.scalar.copy(bandB[pp][:, gi], psB[:])

                if gi != gn - 1:
                    continue

                # ---------------- batched select over the finished group
                def tt(eng, dst, a, b, op):
                    eng.tensor_tensor(dst, a, b, op)

                dv = nc.vector

                # stage-1 select chains: masked direct radix-8, 18-wide
                def sel_chain(l, dst, gs=gs, gn=gn, pp=pp):
                    rows = BH[l]
                    src = band_view(l, pp)
                    p0, p1, q = [p[:, 0:gn, 0:rows, :] for p in pS]
                    tt(dv, p0, src[:, 0:gn, :, 0:18],
                       coefS(gs, gn, S_A[l], rows), MUL)
                    pc = [p0, p1]
                    for a in range(1, NA[l]):
                        tt(dv, q, src[:, 0:gn, :, 8 * a:8 * a + 18],
                           coefS(gs, gn, S_A[l] + 18 * a, rows), MUL)
                        d = dst if a == NA[l] - 1 else pc[a % 2]
                        tt(dv, d, pc[(a + 1) % 2], q, ADD)

                sel_chain(0, s1fL0[:, 0:gn])
                sel_chain(1, s1f123[pp][:, 0:3 * gn:3])
                sel_chain(2, s1f123[pp][:, 1:3 * gn:3])
                sel_chain(3, s1f123[pp][:, 2:3 * gn:3])

                # y select+blend, L0 (4 taps, DVE) -> s2m slots (g, 0)
                za = [p[:, 0:gn, 0:9, :] for p in pS]
                tt(dv, za[0], s1fL0[:, 0:gn, 0:9, :], coefS(gs, gn, S_Y0, 9), MUL)
                for d in range(1, 4):
                    tt(dv, za[2], s1fL0[:, 0:gn, d:d + 9, :],
                       coefS(gs, gn, S_Y0 + 18 * d, 9), MUL)
                    dst = s2m[:, 0:4 * gn:4] if d == 3 else za[d % 2]
                    tt(dv, dst, za[(d + 1) % 2], za[2], ADD)

                # y select+blend, L1-3: 3 tap muls + adds -> s2m slots (g, 1..3)
                s1s = s1f123[pp]
                tt(dv, wv[0][:, 0:3 * gn], s1s[:, 0:3 * gn, 0:9, :],
                   coefY(gs, gn, 0), MUL)
                tt(dv, wv[2][:, 0:3 * gn], s1s[:, 0:3 * gn, 1:10, :],
                   coefY(gs, gn, 18), MUL)
                tt(dv, wv[1][:, 0:3 * gn], wv[0][:, 0:3 * gn], wv[2][:, 0:3 * gn],
                   ADD)
                tt(dv, wv[2][:, 0:3 * gn], s1s[:, 0:3 * gn, 2:11, :],
                   coefY(gs, gn, 36), MUL)
                for l in range(1, 4):
                    tt(dv, s2m[:, l:4 * gn:4],
                       wv[1][:, l - 1:3 * gn:3], wv[2][:, l - 1:3 * gn:3], ADD)

                # fine-x coarse select (radix-3), all levels fused
                tt(dv, qv[0][:, 0:4 * gn], s2m[:, 0:4 * gn, :, 0:12],
                   coefQJ(gs, gn, 0, 12), MUL)
                tt(dv, qv[2][:, 0:4 * gn], s2m[:, 0:4 * gn, :, 3:15],
                   coefQJ(gs, gn, 12, 12), MUL)
                tt(dv, qv[1][:, 0:4 * gn], qv[0][:, 0:4 * gn],
                   qv[2][:, 0:4 * gn], ADD)
                tt(dv, qv[2][:, 0:4 * gn], s2m[:, 0:4 * gn, :, 6:18],
                   coefQJ(gs, gn, 24, 12), MUL)
                tt(dv, qv[0][:, 0:4 * gn], qv[1][:, 0:4 * gn],
                   qv[2][:, 0:4 * gn], ADD)

                # fine-x 4-tap blend chain -> outg
                outg = outp.tile([128, G * 4, 81], bf16, tag="outg", name="outg")
                s3 = qv[0]
                jp = [qv[1][:, 0:4 * gn, :, 0:9], s2m[:, 0:4 * gn, :, 0:9]]
                jqs = qv[2][:, 0:4 * gn, :, 0:9]
                tt(dv, jp[0], s3[:, 0:4 * gn, :, 0:9], coefQJ(gs, gn, 36, 9), MUL)
                odst = outg[:, 0:4 * gn].rearrange("p s (a b) -> p s a b", b=9)
                for j in range(1, 4):
                    tt(dv, jqs, s3[:, 0:4 * gn, :, j:j + 9],
                       coefQJ(gs, gn, 36 + 9 * j, 9), MUL)
                    dst = odst if j == 3 else jp[j % 2]
                    tt(dv, dst, jp[(j + 1) % 2], jqs, ADD)

                nc.sync.dma_start(out_h[:, gs * 4:(gs + gn) * 4],
                                  outg[:, 0:4 * gn])

    _split_waits(nc, mybir)
    return nc


def _split_waits(nc, mybir, limit=1):
    """This container's walrus supports only one sync wait per instruction;
    move extra waits onto same-engine NOPs inserted just before."""
    ctr = [0]
    for f in nc.m.functions:
        for bb in f.blocks:
            out = []
            changed = False
            for inst in bb.instructions:
                si = inst.sync_info
                waits = list(si.on_wait) if (si and si.on_wait) else []
                if len(waits) > limit:
                    si.on_wait = waits[:limit]
                    for w in waits[limit:]:
                        nop = mybir.InstNoOp(
                            name=f"wsplit-{ctr[0]}", ins=[], outs=[])
                        ctr[0] += 1
                        nop.engine = inst.engine
                        nop.sync_info = mybir.SyncInfo(on_wait=[w], on_update=[])
                        out.append(nop)
                    changed = True
                out.append(inst)
            if changed:
                bb.instructions = out
    return nc


def _get_program():
    if "nc" not in _cached:
        _cached["nc"] = _build_program()
    return _cached["nc"]


# ------------------------------------------------------------------ host prep
def _prepare(fmap1, fmap2, centroids_coords):
    f1 = np.asarray(fmap1, np.float32).reshape(B, C, HW)
    f2 = np.asarray(fmap2, np.float32)
    cent = np.asarray(centroids_coords, np.float32)

    # fold 1/sqrt(C) = 1/16 into f2 (exact in fp32)
    f2pools = [_pool_levels(f2[b] / np.float32(16.0)) for b in range(B)]

    cx = cent[:, 0].reshape(B, HW)
    cy = cent[:, 1].reshape(B, HW)

    in_maps = []
    post_cores = []
    prt = np.arange(TP)
    for core in range(NCORES):
        b = core // (NCORES // B)
        qtr = core % (NCORES // B)
        order = np.argsort(cy[b], kind="stable")
        pix = order[qtr * P_CORE:(qtr + 1) * P_CORE].reshape(TILES, TP)
        for t in range(TILES):
            pix[t] = pix[t][np.argsort(cx[b][pix[t]], kind="stable")]
        pixf = pix.reshape(-1)

        f1p = np.ascontiguousarray(
            f1[b][:, pixf].astype(BF16).reshape(2, 128, P_CORE))

        f2band = np.zeros((TILES, 2, 128, NCOLS), BF16)
        repS = np.zeros((TP, TILES, RWS), np.float32)
        repY = np.zeros((TP, TILES * 3, 54), np.float32)
        repQJ = np.zeros((TP, TILES * 4, 72), np.float32)
        fixups = []

        for t in range(TILES):
            tcx = cx[b][pix[t]]
            tcy = cy[b][pix[t]]
            for l in range(NUM_LEVELS):
                sc = np.float32(1 << l)
                lx = tcx / sc
                ly = tcy / sc
                fx = np.floor(lx)
                fy = np.floor(ly)
                x0 = fx.astype(np.int64) - RADIUS
                y0 = fy.astype(np.int64) - RADIUS
                wx1 = (lx - fx).astype(np.float32)
                wy1 = (ly - fy).astype(np.float32)
                r0 = int(y0.min())

                gpx = GPX[l]
                ngrp = TP // gpx
                Xg = np.empty(TP, np.int64)
                bases = []
                for g in range(ngrp):
                    seg = x0[g * gpx:(g + 1) * gpx]
                    base = int(seg.min()) if gpx < TP else -RADIUS
                    bases.append(base)
                    Xg[g * gpx:(g + 1) * gpx] = base
                o = x0 - Xg

                bad = (y0 > r0 + SPAN[l]) | (o > WIN[l] - 18)
                if bad.any():
                    idx = np.nonzero(bad)[0]
                    fixups.append((l, t, idx, lx[idx], ly[idx]))
                good = ~bad

                # band gather per group
                fp = f2pools[b][l]
                h_l = WL[l]
                if l == 0:
                    coff = C_L0
                elif l == 1:
                    coff = C_L1
                elif l == 2:
                    coff = [C_L2]
                else:
                    coff = [C_L3]
                for g in range(ngrp):
                    X = bases[g]
                    blk = np.zeros((C, BH[l], WIN[l]), np.float32)
                    rlo = max(r0, 0)
                    rhi = min(r0 + BH[l], h_l)
                    clo = max(X, 0)
                    chi = min(X + WIN[l], h_l)
                    if rhi > rlo and chi > clo:
                        blk[:, rlo - r0:rhi - r0, clo - X:chi - X] = \
                            fp[:, rlo:rhi, clo:chi]
                    f2band[t, :, :, coff[g]:coff[g] + NL[l]] = \
                        blk.reshape(2, 128, NL[l]).astype(BF16)

                # ---- meta coefficients (zeroed for bad pixels)
                oc = np.clip(o, 0, WIN[l] - 18)
                a = (oc >> 3).astype(np.int64)
                aoh = np.zeros((TP, NA[l]), np.float32)
                aoh[prt[good], a[good]] = 1.0
                repS[:, t, S_A[l]:S_A[l] + NA[l] * 18] = np.repeat(aoh, 18, axis=1)

                yoff = np.clip(y0 - r0, 0, SPAN[l]).astype(np.int64)
                yco = np.zeros((TP, NY[l]), np.float32)
                yco[prt[good], yoff[good]] = (1.0 - wy1)[good]
                yco[prt[good], yoff[good] + 1] = wy1[good]
                if l == 0:
                    repS[:, t, S_Y0:S_Y0 + 72] = np.repeat(yco, 18, axis=1)
                else:
                    repY[:, t * 3 + (l - 1), :] = np.repeat(yco, 18, axis=1)

                xb = (oc & 7).astype(np.int64)
                qi = xb // 3
                rr = xb % 3
                qoh = np.zeros((TP, 3), np.float32)
                qoh[prt[good], qi[good]] = 1.0
                jco = np.zeros((TP, 4), np.float32)
                jco[prt[good], rr[good]] = (1.0 - wx1)[good]
                jco[prt[good], rr[good] + 1] = wx1[good]
                repQJ[:, t * 4 + l, 0:36] = np.repeat(qoh, 12, axis=1)
                repQJ[:, t * 4 + l, 36:72] = np.repeat(jco, 9, axis=1)

        in_maps.append({
            "f1p": f1p,
            "f2band": f2band,
            "repS": np.ascontiguousarray(
                repS.astype(BF16).reshape(128, TILES, 1, RWS)),
            "repY": np.ascontiguousarray(
                repY.astype(BF16).reshape(128, TILES * 3, 1, 54)),
            "repQJ": np.ascontiguousarray(
                repQJ.astype(BF16).reshape(128, TILES * 4, 1, 72)),
        })
        post_cores.append({"b": b, "pix": pixf, "fixups": fixups,
                           "f1b": f1[b], "f2pools": f2pools[b]})
    return in_maps, post_cores


def _assemble(results, post_cores):
    out = np.zeros((B, NUM_LEVELS * K * K, H, W), np.float32)
    for core, (res, pc) in enumerate(zip(results, post_cores)):
        raw = np.asarray(res["out"])
        # device layout: raw[p, t*4+l, a*9+b], a = y-offset idx, b = x-offset idx
        # reference channel = l*81 + i*9 + j with i = x idx, j = y idx
        arr = raw.astype(np.float32).reshape(128, TILES, 4, 9, 9)
        dev = np.ascontiguousarray(
            arr.transpose(1, 0, 2, 4, 3).reshape(P_CORE, NUM_LEVELS * K * K))
        b = pc["b"]
        pix = pc["pix"]

        for (l, t, idx, lx, ly) in pc["fixups"]:
            gpix = pix[t * TP + idx]
            f1cols = pc["f1b"][:, gpix]
            fp = pc["f2pools"][l]
            cmap = np.einsum("cn,chw->nhw", f1cols, fp).astype(np.float32)
            samp = _sample_np(cmap, lx, ly)  # [n, i(x), j(y)]
            dev[t * TP + idx, l * 81:(l + 1) * 81] = samp.reshape(-1, 81)

        py, px = pix // W, pix % W
        out[b, :, py, px] = dev
    return out


# ------------------------------------------------------------------- runner
def _ensure_trace_hook():
    """Inject antenv.axon_hooks + NTFF hook so trace=True works in this image."""
    try:
        import antenv
        if "antenv.axon_hooks" in sys.modules:
            return
        mod = types.ModuleType("antenv.axon_hooks")
        mod._hook = None
        def set_axon_ntff_profile_hook(h):
            mod._hook = h
        def get_axon_ntff_profile_hook():
            return mod._hook
        mod.set_axon_ntff_profile_hook = set_axon_ntff_profile_hook
        mod.get_axon_ntff_profile_hook = get_axon_ntff_profile_hook
        sys.modules["antenv.axon_hooks"] = mod
        antenv.axon_hooks = mod
        from trn_agent_boot.trn_boot import _ntff_profile_via_ctypes
        h = _ntff_profile_via_ctypes("/opt/axon/libaxon_pjrt.so")
        if h is not None:
            set_axon_ntff_profile_hook(h)
    except Exception:
        pass


last_exec_time_ns = None


def kernel(fmap1, fmap2, centroids_coords):
    global last_exec_time_ns
    from concourse.bass_utils import run_bass_kernel_spmd

    trace = bool(int(os.environ.get("CORRBLOCK_TRACE", "0")))
    if trace:
        _ensure_trace_hook()

    nc = _get_program()
    in_maps, post_cores = _prepare(fmap1, fmap2, centroids_coords)
    res = run_bass_kernel_spmd(nc, in_maps, list(range(NCORES)), trace=trace)
    last_exec_time_ns = res.exec_time_ns
    return _assemble(res.results, post_cores)


# revision 25
# speedup vs baseline: 1.1839x; 1.1839x over previous
"""Trainium2 Bass kernel for RAFT-style CorrBlock (all-pairs correlation +
pyramid + 9x9 bilinear window sampling).

Contract: kernel(**inputs) takes FULL inputs (fmap1, fmap2, centroids_coords)
and returns the FULL output (B, NUM_LEVELS*81, H, W) as float32.

Strategy (v2 -- pixel-group band windows + fused level chains)
--------------------------------------------------------------
* avg-pooling the correlation volume == correlating against avg-pooled fmap2
  -> pool fmap2 on host, never materialize the (BHW, H, W) pyramid.
* pixels are sorted by centroid-y into 18 tiles of 128, then by centroid-x
  within each tile.  Per tile the x-sorted pixels are split into pixel
  GROUPS (2x64 at L0, 4x32 at L1); each group gets its own narrow band
  window of (pooled) fmap2, gathered on the host.  Col-tiled matmuls
  (tile_position=(0, 32g/64g)) write every group's correlation band to the
  SAME psum columns on the group's own partitions -- so the per-pixel
  coarse-x select shrinks from a 96-wide radix-16+2 cascade to a short
  direct radix-8 masked chain over an 18-wide window.  L2/L3 maps are small
  enough that a shared full-width band is cheaper.
* all selection/blend stages are uniform 18-wide masked tensor_tensor
  chains batched over up-to-6-tile groups; the y-select and the fine-x
  (q/j) stages are additionally fused ACROSS PYRAMID LEVELS via a
  slot-major [tile-member x level] layout, cutting DVE instruction count
  ~2.4x vs per-level chains (the ~151-cycle per-op DVE overhead was ~30%
  of the baseline's runtime).
* engine split: TensorE runs the col-tiled band matmuls; ACT copies
  PSUM->SBUF band slabs (with the fp32->bf16 cast); DVE runs every
  select/blend chain (gpsimd is kept idle: its SBUF port is shared with
  the DVE and any gpsimd streaming halves DVE throughput); Sync drives
  all DMAs.
"""

import os
import sys
import types

import numpy as np

if "/opt/trn_rl_repo" not in sys.path:
    sys.path.insert(0, "/opt/trn_rl_repo")

import ml_dtypes

BF16 = ml_dtypes.bfloat16

# ----------------------------------------------------------------- constants
B, C, H, W = 2, 256, 96, 96
NUM_LEVELS = 4
RADIUS = 4
K = 2 * RADIUS + 1  # 9
HW = H * W
NCORES = 8
P_CORE = B * HW // NCORES  # 2304 query pixels per core
TP = 128                   # pixels (partitions) per tile
TILES = P_CORE // TP       # 18
G = 6                      # max tiles per batched select group

WL = [96, 48, 24, 12]      # level map widths (== heights)
SPAN = [2, 1, 1, 1]        # max allowed y0 span inside a tile
BH = [12, 11, 11, 11]      # band rows per level
NY = [4, 3, 3, 3]          # y-select taps per level

GPX = [64, 32, 128, 128]   # pixels per x-group (128 => shared band)
WIN = [70, 36, 42, 30]     # band window width per level
NA = [7, 3, 3, 2]          # direct radix-8 select options per level
NL = [BH[l] * WIN[l] for l in range(NUM_LEVELS)]  # [840, 396, 462, 330]

# f2band column layout per (tile, cc): [L1 x4 | L0 x2 | L2 | L3]
C_L1 = [0, 396, 792, 1188]
C_L0 = [1584, 2424]
C_L2 = 3264
C_L3 = 3528
NCOLS = 3660

# psum: psA [128,1352->1536 x2bufs]: L1 [0:396), L0 [512:1352)
#       psB [128,644->1024 x1]: L2 [0:264) (nonzero cols), L3 [512:644)
# repS column layout (stage-1 one-hots + L0 y-coefs), 18-wide runs:
S_A = [0, 126, 180, 234]   # L0 7x18, L1 3x18, L2 3x18, L3 2x18
S_Y0 = 270                 # L0 y: 4x18
RWS = 342
# repY: slot (t, l-1), 3 y taps x 18 = 54
# repQJ: slot (t, l), q 3x12 | j 4x9 = 72

GROUPS = [(0, 2), (2, 4), (6, 6), (12, 6)]

_cached = {}


# ------------------------------------------------------------------- helpers
def _pool_levels(f2_scaled):
    """f2_scaled: (C, H, W) fp32 -> list of (C, H_l, W_l)."""
    out = [f2_scaled]
    cur = f2_scaled
    for _ in range(NUM_LEVELS - 1):
        c, h, w = cur.shape
        cur = cur.reshape(c, h // 2, 2, w // 2, 2).mean(axis=(2, 4), dtype=np.float32)
        out.append(cur)
    return out


def _sample_np(cmap, cx, cy):
    """Reference-equivalent 9x9 bilinear sampling of one level map.

    cmap: (n, h, w) fp32; cx, cy: (n,) absolute coords at this level.
    Returns (n, K, K) with [i, j] = sample at (x=cx+di[i], y=cy+di[j]).
    """
    n, h, w = cmap.shape
    di = np.linspace(-RADIUS, RADIUS, K).astype(np.float32)
    x = np.broadcast_to(cx[:, None, None] + di[None, :, None], (n, K, K))
    y = np.broadcast_to(cy[:, None, None] + di[None, None, :], (n, K, K))
    x0 = np.floor(x)
    y0 = np.floor(y)
    wx1 = x - x0
    wy1 = y - y0
    res = np.zeros((n, K, K), np.float32)
    ni = np.arange(n)[:, None, None]
    for dx, wxt in ((0, 1.0 - wx1), (1, wx1)):
        for dy, wyt in ((0, 1.0 - wy1), (1, wy1)):
            xi = x0 + dx
            yi = y0 + dy
            valid = (xi >= 0) & (xi <= w - 1) & (yi >= 0) & (yi <= h - 1)
            xc = np.clip(xi, 0, w - 1).astype(np.int64)
            yc = np.clip(yi, 0, h - 1).astype(np.int64)
            res += np.where(valid, cmap[ni, yc, xc], 0.0) * wxt * wyt
    return res


# ------------------------------------------------------------- bass program
def _build_program():
    import concourse.bass as bass
    import concourse.tile as tile
    from concourse import mybir
    from concourse.vector_clock import ScopedClock

    # walrus in this container only supports one sync wait on the tail
    # Drain/NoOp -- split the tile tail waits onto single-wait NOPs.
    def _patched_drain_and_barrier(self, tick_clock, wait_clock):
        nc = self.nc
        probe = nc.sync.nop()
        wait_clock.add_sem_waits(probe.ins, ScopedClock({None: tick_clock.global_clock}))
        si = probe.ins.sync_info
        waits = list(si.on_wait or []) if si else []
        if len(waits) > 1:
            si.on_wait = waits[:1]
            for wt in waits[1:]:
                n2 = nc.sync.nop()
                n2.ins.sync_info = mybir.SyncInfo(on_wait=[wt], on_update=[])
        nc.sync.drain()
        nc.all_engine_barrier()
        popped = nc._tile_sem_poison_stack.pop()
        assert popped is self._sem_poison
        nc.clear_and_free_semaphores(list(self.sems.allocated().values()))
        nc.all_engine_barrier()

    tile.TileContext._drain_and_barrier = _patched_drain_and_barrier

    f32 = mybir.dt.float32
    bf16 = mybir.dt.bfloat16
    MUL = mybir.AluOpType.mult
    ADD = mybir.AluOpType.add


    nc = bass.Bass()
    f1_h = nc.declare_dram_parameter("f1p", [2, 128, P_CORE], bf16, isOutput=False)
    f2_h = nc.declare_dram_parameter("f2band", [TILES, 2, 128, NCOLS], bf16,
                                     isOutput=False)
    repS_h = nc.declare_dram_parameter("repS", [128, TILES, 1, RWS], bf16,
                                       isOutput=False)
    repY_h = nc.declare_dram_parameter("repY", [128, TILES * 3, 1, 54], bf16,
                                       isOutput=False)
    repQJ_h = nc.declare_dram_parameter("repQJ", [128, TILES * 4, 1, 72], bf16,
                                        isOutput=False)
    out_h = nc.declare_dram_parameter(
        "out", [128, TILES * 4, 81], bf16, isOutput=True)

    with tile.TileContext(nc) as tc:
        with (
            tc.tile_pool(name="persist", bufs=1) as persist,
            tc.tile_pool(name="f2in", bufs=2) as f2in,
            tc.tile_pool(name="psum", bufs=1, space="PSUM") as psumpool,
            tc.tile_pool(name="outp", bufs=2) as outp,
        ):
            f1sb = [persist.tile([128, P_CORE], bf16, tag=f"f1_{cc}", name=f"f1_{cc}")
                    for cc in range(2)]
            for cc in range(2):
                nc.sync.dma_start(f1sb[cc][:], f1_h[cc])
            repSsb = persist.tile([128, TILES, 1, RWS], bf16, tag="repS", name="repS")
            repYsb = persist.tile([128, TILES * 3, 1, 54], bf16, tag="repY", name="repY")
            repQJsb = persist.tile([128, TILES * 4, 1, 72], bf16, tag="repQJ",
                                   name="repQJ")


            # band slabs, double-buffered per group parity; each slab is a
            # verbatim image of its psum tile (incl. the 512-alignment gaps)
            # so one ACT copy moves a whole tile: A = [L1 0:396 | L0 512:1352),
            # B = [L2 0:462 | L3 512:842)
            bandA = [persist.tile([128, G, 1352], bf16, tag=f"bandA_{pp}",
                                  name=f"bandA_{pp}") for pp in range(2)]
            bandB = [persist.tile([128, G, 842], bf16, tag=f"bandB_{pp}",
                                  name=f"bandB_{pp}") for pp in range(2)]

            for _pp in range(2):
                nc.vector.memset(bandB[_pp][:], 0.0)

            def band_view(l, pp):
                if l == 0:
                    return bandA[pp][:, :, 512:1352].rearrange(
                        "p g (r w) -> p g r w", w=70)
                if l == 1:
                    return bandA[pp][:, :, 0:396].rearrange(
                        "p g (r w) -> p g r w", w=36)
                if l == 2:
                    return bandB[pp][:, :, 0:462].rearrange(
                        "p g (r w) -> p g r w", w=42)
                return bandB[pp][:, :, 512:842].rearrange(
                    "p g (r w) -> p g r w", w=30)
            # stage-1 outputs
            s1fL0 = persist.tile([128, G, 12, 18], bf16, tag="s1fL0", name="s1fL0")
            s1f123 = [persist.tile([128, G * 3, 11, 18], bf16, tag=f"s1f123_{pp}",
                                   name=f"s1f123_{pp}") for pp in range(2)]
            s2m = persist.tile([128, G * 4, 9, 18], bf16, tag="s2m", name="s2m")
            # scratch: L0 select (also reused, row-sliced, by the L0 y chain)
            pS = [persist.tile([128, G, 12, 18], bf16, tag=f"pS{i}", name=f"pS{i}")
                  for i in range(3)]
            # y123 scratch
            wv = [persist.tile([128, G * 3, 9, 18], bf16, tag=f"wv{i}", name=f"wv{i}")
                  for i in range(3)]
            # q scratch (also reused, col-sliced, by the j chain)
            qv = [persist.tile([128, G * 4, 9, 12], bf16, tag=f"qv{i}", name=f"qv{i}")
                  for i in range(3)]

            tile_grp = {}
            for grp, (gs, gn) in enumerate(GROUPS):
                for i in range(gn):
                    tile_grp[gs + i] = (grp, gs, gn, i)

            def coefS(gs, gn, col, rows, w=18):
                return repSsb[:, gs:gs + gn, 0:1, col:col + w].broadcast_to(
                    (128, gn, rows, w))

            def coefY(gs, gn, col):
                return repYsb[:, gs * 3:(gs + gn) * 3, 0:1, col:col + 18].broadcast_to(
                    (128, 3 * gn, 9, 18))

            def coefQJ(gs, gn, col, w):
                return repQJsb[:, gs * 4:(gs + gn) * 4, 0:1, col:col + w].broadcast_to(
                    (128, 4 * gn, 9, w))

            for t in range(TILES):
                grp, gs, gn, gi = tile_grp[t]
                pp = grp % 2
                f2sb = [f2in.tile([128, NCOLS], bf16, tag=f"f2_{cc}",
                                  name=f"f2sb_{cc}") for cc in range(2)]
                for cc in range(2):
                    nc.sync.dma_start(f2sb[cc][:], f2_h[t, cc])
                if t == 0:
                    nc.sync.dma_start(repSsb[:], repS_h[:])
                elif t == 1:
                    nc.sync.dma_start(repYsb[:], repY_h[:])
                    nc.sync.dma_start(repQJsb[:], repQJ_h[:])

                psA = psumpool.tile([128, 1352], f32, tag="psA", name="psA", bufs=2)
                psB = psumpool.tile([128, 644], f32, tag="psB", name="psB")
                for cc in range(2):
                    st = (cc == 0)
                    sp = (cc == 1)
                    for g in range(4):  # L1 groups of 32
                        lhsT = f1sb[cc][:, t * TP + g * 32:t * TP + (g + 1) * 32]
                        nc.tensor.matmul(
                            psA[g * 32:(g + 1) * 32, 0:396], lhsT,
                            f2sb[cc][:, C_L1[g]:C_L1[g] + 396],
                            start=st, stop=sp, tile_position=(0, g * 32))
                    for g in range(2):  # L0 groups of 64
                        lhsT = f1sb[cc][:, t * TP + g * 64:t * TP + (g + 1) * 64]
                        nc.tensor.matmul(
                            psA[g * 64:(g + 1) * 64, 512:1024], lhsT,
                            f2sb[cc][:, C_L0[g]:C_L0[g] + 512],
                            start=st, stop=sp, tile_position=(0, g * 64))
                        nc.tensor.matmul(
                            psA[g * 64:(g + 1) * 64, 1024:1352], lhsT,
                            f2sb[cc][:, C_L0[g] + 512:C_L0[g] + 840],
                            start=st, stop=sp, tile_position=(0, g * 64))
                    lhsT = f1sb[cc][:, t * TP:(t + 1) * TP]
                    nc.tensor.matmul(psB[:, 0:264], lhsT,
                                     f2sb[cc][:, C_L2:C_L2 + 264],
                                     start=st, stop=sp)
                    nc.tensor.matmul(psB[:, 512:644], lhsT,
                                     f2sb[cc][:, C_L3:C_L3 + 132],
                                     start=st, stop=sp)

                # PSUM -> SBUF band slabs (fp32 -> bf16 on ACT), one copy
                # per psum tile
                nc.scalar.copy(bandA[pp][:, gi], psA[:])
                nc.scalar.copy(
                    bandB[pp][:, gi, 0:462].rearrange(
                        "p (r w) -> p r w", w=42)[:, :, 4:28],
                    psB[:, 0:264].rearrange("p (r w) -> p r w", w=24))
                nc.scalar.copy(
                    bandB[pp][:, gi, 512:842].rearrange(
                        "p (r w) -> p r w", w=30)[:, :, 4:16],
                    psB[:, 512:644].rearrange("p (r w) -> p r w", w=12))

                if gi != gn - 1:
                    continue

                # ---------------- batched select over the finished group
                def tt(eng, dst, a, b, op):
                    eng.tensor_tensor(dst, a, b, op)

                dv = nc.vector

                # stage-1 select chains: masked direct radix-8, 18-wide
                def sel_chain(l, dst, gs=gs, gn=gn, pp=pp):
                    rows = BH[l]
                    src = band_view(l, pp)
                    p0, p1, q = [p[:, 0:gn, 0:rows, :] for p in pS]
                    tt(dv, p0, src[:, 0:gn, :, 0:18],
                       coefS(gs, gn, S_A[l], rows), MUL)
                    pc = [p0, p1]
                    for a in range(1, NA[l]):
                        tt(dv, q, src[:, 0:gn, :, 8 * a:8 * a + 18],
                           coefS(gs, gn, S_A[l] + 18 * a, rows), MUL)
                        d = dst if a == NA[l] - 1 else pc[a % 2]
                        tt(dv, d, pc[(a + 1) % 2], q, ADD)

                sel_chain(0, s1fL0[:, 0:gn])
                sel_chain(1, s1f123[pp][:, 0:3 * gn:3])
                sel_chain(2, s1f123[pp][:, 1:3 * gn:3])
                sel_chain(3, s1f123[pp][:, 2:3 * gn:3])

                # y select+blend, L0 (4 taps, DVE) -> s2m slots (g, 0)
                za = [p[:, 0:gn, 0:9, :] for p in pS]
                tt(dv, za[0], s1fL0[:, 0:gn, 0:9, :], coefS(gs, gn, S_Y0, 9), MUL)
                for d in range(1, 4):
                    tt(dv, za[2], s1fL0[:, 0:gn, d:d + 9, :],
                       coefS(gs, gn, S_Y0 + 18 * d, 9), MUL)
                    dst = s2m[:, 0:4 * gn:4] if d == 3 else za[d % 2]
                    tt(dv, dst, za[(d + 1) % 2], za[2], ADD)

                # y select+blend, L1-3: 3 tap muls + adds -> s2m slots (g, 1..3)
                s1s = s1f123[pp]
                tt(dv, wv[0][:, 0:3 * gn], s1s[:, 0:3 * gn, 0:9, :],
                   coefY(gs, gn, 0), MUL)
                tt(dv, wv[2][:, 0:3 * gn], s1s[:, 0:3 * gn, 1:10, :],
                   coefY(gs, gn, 18), MUL)
                tt(dv, wv[1][:, 0:3 * gn], wv[0][:, 0:3 * gn], wv[2][:, 0:3 * gn],
                   ADD)
                tt(dv, wv[2][:, 0:3 * gn], s1s[:, 0:3 * gn, 2:11, :],
                   coefY(gs, gn, 36), MUL)
                for l in range(1, 4):
                    tt(dv, s2m[:, l:4 * gn:4],
                       wv[1][:, l - 1:3 * gn:3], wv[2][:, l - 1:3 * gn:3], ADD)

                # fine-x coarse select (radix-3), all levels fused
                tt(dv, qv[0][:, 0:4 * gn], s2m[:, 0:4 * gn, :, 0:12],
                   coefQJ(gs, gn, 0, 12), MUL)
                tt(dv, qv[2][:, 0:4 * gn], s2m[:, 0:4 * gn, :, 3:15],
                   coefQJ(gs, gn, 12, 12), MUL)
                tt(dv, qv[1][:, 0:4 * gn], qv[0][:, 0:4 * gn],
                   qv[2][:, 0:4 * gn], ADD)
                tt(dv, qv[2][:, 0:4 * gn], s2m[:, 0:4 * gn, :, 6:18],
                   coefQJ(gs, gn, 24, 12), MUL)
                tt(dv, qv[0][:, 0:4 * gn], qv[1][:, 0:4 * gn],
                   qv[2][:, 0:4 * gn], ADD)

                # fine-x 4-tap blend chain -> outg
                outg = outp.tile([128, G * 4, 81], bf16, tag="outg", name="outg")
                s3 = qv[0]
                jp = [qv[1][:, 0:4 * gn, :, 0:9], s2m[:, 0:4 * gn, :, 0:9]]
                jqs = qv[2][:, 0:4 * gn, :, 0:9]
                tt(dv, jp[0], s3[:, 0:4 * gn, :, 0:9], coefQJ(gs, gn, 36, 9), MUL)
                odst = outg[:, 0:4 * gn].rearrange("p s (a b) -> p s a b", b=9)
                for j in range(1, 4):
                    tt(dv, jqs, s3[:, 0:4 * gn, :, j:j + 9],
                       coefQJ(gs, gn, 36 + 9 * j, 9), MUL)
                    dst = odst if j == 3 else jp[j % 2]
                    tt(dv, dst, jp[(j + 1) % 2], jqs, ADD)

                nc.sync.dma_start(out_h[:, gs * 4:(gs + gn) * 4],
                                  outg[:, 0:4 * gn])

    _split_waits(nc, mybir)
    return nc


def _split_waits(nc, mybir, limit=1):
    """This container's walrus supports only one sync wait per instruction;
    move extra waits onto same-engine NOPs inserted just before."""
    ctr = [0]
    for f in nc.m.functions:
        for bb in f.blocks:
            out = []
            changed = False
            for inst in bb.instructions:
                si = inst.sync_info
                waits = list(si.on_wait) if (si and si.on_wait) else []
                if len(waits) > limit:
                    si.on_wait = waits[:limit]
                    for w in waits[limit:]:
                        nop = mybir.InstNoOp(
                            name=f"wsplit-{ctr[0]}", ins=[], outs=[])
                        ctr[0] += 1
                        nop.engine = inst.engine
                        nop.sync_info = mybir.SyncInfo(on_wait=[w], on_update=[])
                        out.append(nop)
                    changed = True
                out.append(inst)
            if changed:
                bb.instructions = out
    return nc


def _get_program():
    if "nc" not in _cached:
        _cached["nc"] = _build_program()
    return _cached["nc"]


# ------------------------------------------------------------------ host prep
def _prepare(fmap1, fmap2, centroids_coords):
    f1 = np.asarray(fmap1, np.float32).reshape(B, C, HW)
    f2 = np.asarray(fmap2, np.float32)
    cent = np.asarray(centroids_coords, np.float32)

    # fold 1/sqrt(C) = 1/16 into f2 (exact in fp32)
    f2pools = [_pool_levels(f2[b] / np.float32(16.0)) for b in range(B)]

    cx = cent[:, 0].reshape(B, HW)
    cy = cent[:, 1].reshape(B, HW)

    in_maps = []
    post_cores = []
    prt = np.arange(TP)
    for core in range(NCORES):
        b = core // (NCORES // B)
        qtr = core % (NCORES // B)
        order = np.argsort(cy[b], kind="stable")
        pix = order[qtr * P_CORE:(qtr + 1) * P_CORE].reshape(TILES, TP)
        for t in range(TILES):
            pix[t] = pix[t][np.argsort(cx[b][pix[t]], kind="stable")]
        pixf = pix.reshape(-1)

        f1p = np.ascontiguousarray(
            f1[b][:, pixf].astype(BF16).reshape(2, 128, P_CORE))

        f2band = np.zeros((TILES, 2, 128, NCOLS), BF16)
        repS = np.zeros((TP, TILES, RWS), np.float32)
        repY = np.zeros((TP, TILES * 3, 54), np.float32)
        repQJ = np.zeros((TP, TILES * 4, 72), np.float32)
        fixups = []

        for t in range(TILES):
            tcx = cx[b][pix[t]]
            tcy = cy[b][pix[t]]
            for l in range(NUM_LEVELS):
                sc = np.float32(1 << l)
                lx = tcx / sc
                ly = tcy / sc
                fx = np.floor(lx)
                fy = np.floor(ly)
                x0 = fx.astype(np.int64) - RADIUS
                y0 = fy.astype(np.int64) - RADIUS
                wx1 = (lx - fx).astype(np.float32)
                wy1 = (ly - fy).astype(np.float32)
                r0 = int(y0.min())

                gpx = GPX[l]
                ngrp = TP // gpx
                Xg = np.empty(TP, np.int64)
                bases = []
                for g in range(ngrp):
                    seg = x0[g * gpx:(g + 1) * gpx]
                    base = int(seg.min()) if gpx < TP else -RADIUS
                    bases.append(base)
                    Xg[g * gpx:(g + 1) * gpx] = base
                o = x0 - Xg

                bad = (y0 > r0 + SPAN[l]) | (o > WIN[l] - 18)
                if bad.any():
                    idx = np.nonzero(bad)[0]
                    fixups.append((l, t, idx, lx[idx], ly[idx]))
                good = ~bad

                # band gather per group
                fp = f2pools[b][l]
                h_l = WL[l]
                if l == 0:
                    coff = C_L0
                elif l == 1:
                    coff = C_L1
                elif l == 2:
                    coff = [C_L2]
                else:
                    coff = [C_L3]
                for g in range(ngrp):
                    X = bases[g]
                    blk = np.zeros((C, BH[l], WIN[l]), np.float32)
                    rlo = max(r0, 0)
                    rhi = min(r0 + BH[l], h_l)
                    clo = max(X, 0)
                    chi = min(X + WIN[l], h_l)
                    if rhi > rlo and chi > clo:
                        blk[:, rlo - r0:rhi - r0, clo - X:chi - X] = \
                            fp[:, rlo:rhi, clo:chi]
                    if l >= 2:
                        nz = blk[:, :, 4:4 + WL[l]]
                        f2band[t, :, :, coff[g]:coff[g] + 11 * WL[l]] = \
                            nz.reshape(2, 128, 11 * WL[l]).astype(BF16)
                    else:
                        f2band[t, :, :, coff[g]:coff[g] + NL[l]] = \
                            blk.reshape(2, 128, NL[l]).astype(BF16)

                # ---- meta coefficients (zeroed for bad pixels)
                oc = np.clip(o, 0, WIN[l] - 18)
                a = (oc >> 3).astype(np.int64)
                aoh = np.zeros((TP, NA[l]), np.float32)
                aoh[prt[good], a[good]] = 1.0
                repS[:, t, S_A[l]:S_A[l] + NA[l] * 18] = np.repeat(aoh, 18, axis=1)

                yoff = np.clip(y0 - r0, 0, SPAN[l]).astype(np.int64)
                yco = np.zeros((TP, NY[l]), np.float32)
                yco[prt[good], yoff[good]] = (1.0 - wy1)[good]
                yco[prt[good], yoff[good] + 1] = wy1[good]
                if l == 0:
                    repS[:, t, S_Y0:S_Y0 + 72] = np.repeat(yco, 18, axis=1)
                else:
                    repY[:, t * 3 + (l - 1), :] = np.repeat(yco, 18, axis=1)

                xb = (oc & 7).astype(np.int64)
                qi = xb // 3
                rr = xb % 3
                qoh = np.zeros((TP, 3), np.float32)
                qoh[prt[good], qi[good]] = 1.0
                jco = np.zeros((TP, 4), np.float32)
                jco[prt[good], rr[good]] = (1.0 - wx1)[good]
                jco[prt[good], rr[good] + 1] = wx1[good]
                repQJ[:, t * 4 + l, 0:36] = np.repeat(qoh, 12, axis=1)
                repQJ[:, t * 4 + l, 36:72] = np.repeat(jco, 9, axis=1)

        in_maps.append({
            "f1p": f1p,
            "f2band": f2band,
            "repS": np.ascontiguousarray(
                repS.astype(BF16).reshape(128, TILES, 1, RWS)),
            "repY": np.ascontiguousarray(
                repY.astype(BF16).reshape(128, TILES * 3, 1, 54)),
            "repQJ": np.ascontiguousarray(
                repQJ.astype(BF16).reshape(128, TILES * 4, 1, 72)),
        })
        post_cores.append({"b": b, "pix": pixf, "fixups": fixups,
                           "f1b": f1[b], "f2pools": f2pools[b]})
    return in_maps, post_cores


def _assemble(results, post_cores):
    out = np.zeros((B, NUM_LEVELS * K * K, H, W), np.float32)
    for core, (res, pc) in enumerate(zip(results, post_cores)):
        raw = np.asarray(res["out"])
        # device layout: raw[p, t*4+l, a*9+b], a = y-offset idx, b = x-offset idx
        # reference channel = l*81 + i*9 + j with i = x idx, j = y idx
        arr = raw.astype(np.float32).reshape(128, TILES, 4, 9, 9)
        dev = np.ascontiguousarray(
            arr.transpose(1, 0, 2, 4, 3).reshape(P_CORE, NUM_LEVELS * K * K))
        b = pc["b"]
        pix = pc["pix"]

        for (l, t, idx, lx, ly) in pc["fixups"]:
            gpix = pix[t * TP + idx]
            f1cols = pc["f1b"][:, gpix]
            fp = pc["f2pools"][l]
            cmap = np.einsum("cn,chw->nhw", f1cols, fp).astype(np.float32)
            samp = _sample_np(cmap, lx, ly)  # [n, i(x), j(y)]
            dev[t * TP + idx, l * 81:(l + 1) * 81] = samp.reshape(-1, 81)

        py, px = pix // W, pix % W
        out[b, :, py, px] = dev
    return out


# ------------------------------------------------------------------- runner
def _ensure_trace_hook():
    """Inject antenv.axon_hooks + NTFF hook so trace=True works in this image."""
    try:
        import antenv
        if "antenv.axon_hooks" in sys.modules:
            return
        mod = types.ModuleType("antenv.axon_hooks")
        mod._hook = None
        def set_axon_ntff_profile_hook(h):
            mod._hook = h
        def get_axon_ntff_profile_hook():
            return mod._hook
        mod.set_axon_ntff_profile_hook = set_axon_ntff_profile_hook
        mod.get_axon_ntff_profile_hook = get_axon_ntff_profile_hook
        sys.modules["antenv.axon_hooks"] = mod
        antenv.axon_hooks = mod
        from trn_agent_boot.trn_boot import _ntff_profile_via_ctypes
        h = _ntff_profile_via_ctypes("/opt/axon/libaxon_pjrt.so")
        if h is not None:
            set_axon_ntff_profile_hook(h)
    except Exception:
        pass


last_exec_time_ns = None


def kernel(fmap1, fmap2, centroids_coords):
    global last_exec_time_ns
    from concourse.bass_utils import run_bass_kernel_spmd

    trace = bool(int(os.environ.get("CORRBLOCK_TRACE", "0")))
    if trace:
        _ensure_trace_hook()

    nc = _get_program()
    in_maps, post_cores = _prepare(fmap1, fmap2, centroids_coords)
    res = run_bass_kernel_spmd(nc, in_maps, list(range(NCORES)), trace=trace)
    last_exec_time_ns = res.exec_time_ns
    return _assemble(res.results, post_cores)
